# revision 1
# baseline (speedup 1.0000x reference)
"""ChromosomeEmbedding kernel for 8x Trainium2 NeuronCores.

Computes out[b, j, d] = ce[chr[b]-1, d] for b in [0,512), j in [0,2001),
d in [0,128). Data-parallel: the batch is sharded 64 samples/core across
8 cores; the 24x128 table lookup (64 rows -> 32 KB) is folded into host
input prep, so the device program is a pure HBM-write streamer. The
per-core output shard is 65.5 MB; the measured per-core HBM write cap is
~336 GB/s (all 16 SDMA engines ~100% busy), so everything here is about
keeping both HWDGE rings saturated from the first microsecond on.

Per-core device program (identical SPMD program on all cores), written
in raw bacc (no TileContext -- avoids Tile's kernel-tail drain + EVSEM
butterfly barriers):
  1. Each HWDGE ring loads its own half of the replica seed: rows
     pre-replicated to 8 bins on the host -> pre[128, 8, 128] (256 KB),
     partitions 0:64 via the sync ring, 64:128 via the scalar ring, so
     neither ring waits on the other before streaming.
  2. Three doubling copies on the vector engine extend 8 -> 64 bins
     (rep[128, 64, 128], 32 KB/partition).
  3. Each ring opens with a ladder (8, 8, 16, 32, 64 bins) whose rungs
     only need the replica width already available -- output bytes start
     flowing ~2 us into the kernel at >=4 KB/descriptor -- then streams
     64-bin 2 MB DMAs back-to-back with no inter-DMA throttling (the
     ring FIFO provides backpressure). Sync walks bins [0, SPLIT),
     scalar [SPLIT, 2001); SPLIT balances the two rings' drain times.
  4. Minimal tail: sync/scalar wait for their ring's completion count,
     bump a done-sem; gpsimd then resets DMA state and clears all sems
     (so the NEFF can re-execute), with no all-engine barriers.
"""

import functools

import numpy as np

from concourse import bacc, mybir
from concourse.bass_utils import run_bass_kernel_spmd

N_CORES = 8
BS = 512
BPC = BS // N_CORES  # 64 samples per core
NBIN = 2001
DIM = 128
N_CHR = 24
REP = 64  # replicated copies of each row held in SBUF
W0 = 8  # host-side pre-replication width (bins) in the input tensor
SPLIT = 1001  # bins walked by the sync ring; scalar ring takes the rest
F32 = mybir.dt.float32


def _ring_plan(nbins):
    """Ladder + steady chunks covering [0, nbins): list of (offset, width,
    min_vsem) in dispatch order. min_vsem v means the DMA sources
    rep[:, 0:width], which needs v doubling copies retired (v=0 -> only
    this ring's input DMA)."""
    plan = []
    off = 0
    ladder = [(W0, 0), (W0, 0), (2 * W0, 1), (4 * W0, 2), (8 * W0, 3)]
    for w, v in ladder:
        if off >= nbins:
            return plan
        w = min(w, nbins - off)
        plan.append((off, w, v))
        off += w
    while off < nbins:
        w = min(REP, nbins - off)
        plan.append((off, w, 3))
        off += w
    return plan


@functools.lru_cache(maxsize=1)
def build_nc():
    nc = bacc.Bacc("TRN2", target_bir_lowering=False)

    pre_h = nc.declare_dram_parameter("pre", [128, W0, DIM], F32, isOutput=False)
    out_h = nc.declare_dram_parameter("out", [BPC, NBIN, DIM], F32, isOutput=True)

    with (
        nc.sbuf_tensor("rep", [128, REP, DIM], F32) as rep,
        nc.semaphore("ssem") as ssem,  # sync-ring DMA completions
        nc.semaphore("asem") as asem,  # scalar-ring DMA completions
        nc.semaphore("vsem") as vsem,  # doubling-copy completions
        nc.semaphore("done") as done,  # ring-drained markers
    ):
        sync_plan = _ring_plan(SPLIT)
        scal_plan = [(SPLIT + o, w, v) for (o, w, v) in _ring_plan(NBIN - SPLIT)]

        # Each ring loads its own partition half of the replica seed.
        nc.sync.dma_start(out=rep[0:64, 0:W0, :], in_=pre_h[0:64, :, :]).then_inc(
            ssem, 16
        )
        nc.scalar.dma_start(out=rep[64:128, 0:W0, :], in_=pre_h[64:128, :, :]).then_inc(
            asem, 16
        )

        # Vector engine: doubling replication W0 -> REP bins (needs both
        # halves loaded).
        nc.vector.wait_ge(ssem, 16)
        nc.vector.wait_ge(asem, 16)
        w = W0
        while w < REP:
            nc.vector.tensor_copy(
                out=rep[:, w : 2 * w, :], in_=rep[:, 0:w, :]
            ).then_inc(vsem, 1)
            w *= 2

        # Sync ring: bins [0, SPLIT) from partitions 0:64.
        nc.sync.wait_ge(ssem, 16)
        seen_v = 0
        for off, wd, v in sync_plan:
            if v > seen_v:
                nc.sync.wait_ge(vsem, v)
                seen_v = v
            nc.sync.dma_start(
                out=out_h[:, off : off + wd, :], in_=rep[0:BPC, 0:wd, :]
            ).then_inc(ssem, 16)

        # Scalar ring: bins [SPLIT, NBIN) from partitions 64:128.
        nc.scalar.wait_ge(asem, 16)
        seen_v = 0
        for off, wd, v in scal_plan:
            if v > seen_v:
                nc.scalar.wait_ge(vsem, v)
                seen_v = v
            nc.scalar.dma_start(
                out=out_h[:, off : off + wd, :], in_=rep[BPC:128, 0:wd, :]
            ).then_inc(asem, 16)

        # Tail: wait for both rings to drain, then restore sem state so
        # the NEFF can be re-executed (sems are only load-time zeroed).
        nc.sync.wait_ge(ssem, 16 * (1 + len(sync_plan)))
        nc.sync.sem_inc(done, 1)
        nc.scalar.wait_ge(asem, 16 * (1 + len(scal_plan)))
        nc.scalar.sem_inc(done, 1)

        nc.gpsimd.wait_ge(done, 2)
        nums = sorted(s.num for s in (ssem, asem, vsem, done))
        lo, hi = nums[0], nums[-1]
        if nums == list(range(lo, hi + 1)):
            ranges = [range(lo, hi + 1)]
        else:
            ranges = [range(n, n + 1) for n in nums]
        for r in ranges:
            nc.gpsimd.dma_reset(r)
            nc.gpsimd.sem_clear(r)

    nc.compile()
    return nc


def make_in_maps(chr_full: np.ndarray, ce: np.ndarray):
    ce_f32 = np.asarray(ce, dtype=np.float32)
    idx = np.asarray(chr_full).astype(np.int64) - 1
    maps = []
    for c in range(N_CORES):
        rows = ce_f32[idx[c * BPC : (c + 1) * BPC]]  # [64, 128]
        both = np.concatenate([rows, rows], axis=0)  # [128, 128]
        pre = np.repeat(both[:, None, :], W0, axis=1)  # [128, W0, 128]
        maps.append({"pre": np.ascontiguousarray(pre)})
    return maps


def kernel(tensor=None, chr=None, ce=None, **_unused):
    chr_np = np.asarray(chr)
    ce_np = np.asarray(ce)
    nc = build_nc()
    res = run_bass_kernel_spmd(
        nc, make_in_maps(chr_np, ce_np), core_ids=list(range(N_CORES))
    )
    out = np.concatenate([r["out"] for r in res.results], axis=0)
    return out.astype(np.float32)

